# revision 4
# baseline (speedup 1.0000x reference)
"""Trainium2 Bass kernel for empirical CRPS loss (mean reduction).

Problem: forecasts (N=20, B=4, C=1, D=12, H=256, W=256) f32, target (B,C,D,H,W) f32.
CRPS = mean_px [ (1/N) sum_i |x_i - y| - (1/N^2) sum_{i<j} |x_i - x_j| ]

Sort-free reduction identities (exact, up to fp ties):
  sum_{i<j} |x_i - x_j| = 2*sum_{i<j} max(x_i,x_j) - (N-1)*sum_i x_i
  sum_i |x_i - y|       = sum_i x_i + N*y - 2*sum_i min(x_i, y)
so the kernel only needs four global sums:
  M  = sum over pairs of max   (DVE scalar_tensor_tensor bypass/max + accum_out,
                                shifted sample blocks)
  Mn = sum_i min(x_i, y)       (same with stride-0 target broadcast)
  S  = sum of all x            (ACT f32->f16 convert, fused accum_out)
  Y  = sum of all y            (ACT f32->f16 convert, fused accum_out)
  sum_px CRPS_px = (39/400)*S + Y - Mn/10 - M/200          [N=20]

Sharding: pure data parallel over pixels, 8 cores x 393216 px.
Each instruction writes its partial sum to its own accumulator column; the
host sums the [128, cols] partials from all cores in float64.
"""
import numpy as np

N = 20
P_TOTAL = 4 * 1 * 12 * 256 * 256   # 3145728 pixels
N_CORES = 8
P_CORE = P_TOTAL // N_CORES        # 393216
FB = 512                           # pixel columns per partition per tile
PT = 128 * FB                      # pixels per tile
NT = P_CORE // PT                  # tiles per core (6)

# accumulator column layout, per tile: 19 M-shift slots, 1 Mn, 1 Sx, 1 Sy
CPT = 22                           # columns per tile
ACC_COLS = CPT * NT

_CACHE = {}


def _build_nc(p_core=P_CORE, fb=FB, nt=NT):
    import concourse.bacc as bacc
    import concourse.mybir as mybir
    from concourse.tile import TileContext
    from concourse.ap import AP

    F32 = mybir.dt.float32
    WDT = mybir.dt.float16
    FB_, NT_, PT_ = fb, nt, 128 * fb
    assert p_core == PT_ * NT_

    nc = bacc.Bacc()
    fc = nc.declare_dram_parameter("forecasts", [N, p_core], F32, isOutput=False)
    tg = nc.declare_dram_parameter("target", [p_core], F32, isOutput=False)
    out = nc.declare_dram_parameter("partials", [128, CPT * NT_], F32, isOutput=True)

    with TileContext(nc) as tc:
        with (
            tc.tile_pool(name="io", bufs=2) as iop,
            tc.tile_pool(name="wk", bufs=2) as wkp,
            tc.tile_pool(name="scr", bufs=1) as scrp,
            tc.tile_pool(name="acc", bufs=1) as accp,
        ):
            accbuf = accp.tile([128, CPT * NT_], F32)
            scratch = scrp.tile([128, N * FB_], WDT)

            FB, NT, PT = FB_, NT_, PT_
            for t in range(NT):
                p0 = t * PT
                c0 = t * CPT
                xt = iop.tile([128, N * FB], F32, tag="x32")
                yt = iop.tile([128, FB], F32, tag="y32")
                # one DMA for the whole x tile: dst iterates [128p][20][FB],
                # src [128 (stride FB)][20 (stride p_core)][FB (stride 1)]
                fap = fc[:, :]
                src = AP(fap.tensor, p0, [[FB, 128], [p_core, N], [1, FB]])
                nc.sync.dma_start(xt.rearrange("p (n f) -> p n f", n=N), src)
                nc.sync.dma_start(yt, tg[p0:p0 + PT].rearrange("(p f) -> p f", p=128))

                xb = wkp.tile([128, N * FB], WDT, tag="xb")
                yb = wkp.tile([128, FB], WDT, tag="yb")
                nc.scalar.activation(xb, xt, mybir.ActivationFunctionType.Copy,
                                     accum_out=accbuf[:, c0 + 20:c0 + 21])
                nc.scalar.activation(yb, yt, mybir.ActivationFunctionType.Copy,
                                     accum_out=accbuf[:, c0 + 21:c0 + 22])

                # M: sum over all pairs (i, i+k) of max(x_i, x_{i+k})
                for k in range(1, N):
                    nblk = N - k
                    nc.vector.scalar_tensor_tensor(
                        out=scratch[:, :nblk * FB],
                        in0=xb[:, :nblk * FB],
                        scalar=0.0,
                        in1=xb[:, k * FB:(k + nblk) * FB],
                        op0=mybir.AluOpType.bypass,
                        op1=mybir.AluOpType.max,
                        accum_out=accbuf[:, c0 + k - 1:c0 + k],
                    )
                # Mn: sum_i min(x_i, y); target broadcast via stride-0 middle dim
                yap = yb[:, :]
                yb3 = AP(yap.tensor, yap.offset,
                         [list(yap.ap[0]), [0, N], list(yap.ap[1])])
                nc.vector.scalar_tensor_tensor(
                    out=scratch[:, :N * FB].rearrange("p (n f) -> p n f", n=N),
                    in0=xb.rearrange("p (n f) -> p n f", n=N),
                    scalar=0.0,
                    in1=yb3,
                    op0=mybir.AluOpType.bypass,
                    op1=mybir.AluOpType.min,
                    accum_out=accbuf[:, c0 + 19:c0 + 20],
                )

            nc.sync.dma_start(out[:, :], accbuf[:, :])
    nc.compile()
    return nc


def _combine(partials_list):
    tot = 0.0
    for p in partials_list:
        p = np.asarray(p, dtype=np.float64).reshape(128, NT, CPT)
        M = p[:, :, 0:19].sum()
        Mn = p[:, :, 19].sum()
        S = p[:, :, 20].sum()
        Y = p[:, :, 21].sum()
        tot += (39.0 / 400.0) * S + Y - Mn / 10.0 - M / 200.0
    return tot / P_TOTAL


def _run(forecasts, target, trace=False):
    from concourse.bass_utils import run_bass_kernel_spmd

    nc = _CACHE.get("nc")
    if nc is None:
        nc = _build_nc()
        _CACHE["nc"] = nc

    fcf = np.asarray(forecasts, dtype=np.float32).reshape(N, P_TOTAL)
    tgf = np.asarray(target, dtype=np.float32).reshape(P_TOTAL)
    in_maps = []
    for c in range(N_CORES):
        sl = slice(c * P_CORE, (c + 1) * P_CORE)
        in_maps.append({
            "forecasts": np.ascontiguousarray(fcf[:, sl]),
            "target": np.ascontiguousarray(tgf[sl]),
        })
    res = run_bass_kernel_spmd(nc, in_maps, list(range(N_CORES)), trace=trace)
    val = _combine([r["partials"] for r in res.results])
    return np.array(val, dtype=np.float32), res


def kernel(forecasts, target):
    val, _ = _run(forecasts, target)
    return val


# revision 8
# speedup vs baseline: 1.5592x; 1.5592x over previous
"""Trainium2 Bass kernel for empirical CRPS loss (mean reduction).

Problem: forecasts (N=20, B=4, C=1, D=12, H=256, W=256) f32, target (B,C,D,H,W) f32.
CRPS = mean_px [ (1/N) sum_i |x_i - y| - (1/N^2) sum_{i<j} |x_i - x_j| ]

Per pixel, with sorted samples X_(0..19):
  sum_{i<j} |x_i - x_j| = sum_k (2k-19) X_(k)        (order-statistic identity)
  sum_i |x_i - y|       = S_px + 20 y - 2 sum_i min(x_i, y)
Both are linear in per-order-statistic column sums, so the kernel:
  1. converts f32 -> f16 on ScalarE (accum_out gives S and Y for free),
  2. sorts the 20 sample blocks per pixel with a verified 97-comparator
     network: VectorE tensor_tensor min/max in f16 (2x perf mode), using a
     21-slot buffer with slot rotation (max -> free slot, min -> in place),
  3. computes min(x_i, y) with a stride-0 broadcast of the target,
  4. reduces each sorted column block / the min blocks to per-partition
     scalars (ScalarE Copy accum_out, DVE tensor_scalar accum),
  5. host combines all per-core [128, cols] partials in float64.

Sharding: pure data parallel over pixels, 8 cores x 393216 px.
"""
import numpy as np

N = 20
P_TOTAL = 4 * 1 * 12 * 256 * 256   # 3145728 pixels
N_CORES = 8
P_CORE = P_TOTAL // N_CORES        # 393216
FB = 1024                          # pixel columns per partition per tile
PT = 128 * FB                      # pixels per tile
NT = P_CORE // PT                  # tiles per core (3)

CVT_CHUNK = 4                      # sample blocks converted per ACT op
MN_CHUNK = 5                       # sample blocks per min/sum chunk

# accumulator columns per tile: 20 sorted col sums, NMN mn sums,
# N/CVT_CHUNK Sx sums, 1 Sy
NMN = N // MN_CHUNK
NCVT = N // CVT_CHUNK
CPT = N + NMN + NCVT + 1

_CACHE = {}

# --- sorting network (97 comparators, verified by 0-1 principle) -----------
SORT5 = [(0, 1), (3, 4), (2, 4), (2, 3), (1, 4), (0, 3), (0, 2), (1, 3), (1, 2)]


def _oe_merge(a, b, net):
    n, m = len(a), len(b)
    if n == 0 or m == 0:
        return
    if n == 1 and m == 1:
        net.append((a[0], b[0]))
        return
    _oe_merge(a[::2], b[::2], net)
    _oe_merge(a[1::2], b[1::2], net)
    c = list(a) + list(b)
    for i in range(1, n + m - 1, 2):
        net.append((c[i], c[i + 1]))


def _sort_net(wires, net):
    n = len(wires)
    if n <= 1:
        return
    if n == 2:
        net.append((wires[0], wires[1]))
        return
    if n == 5:
        for i, j in SORT5:
            net.append((wires[i], wires[j]))
        return
    h = n // 2
    _sort_net(wires[:h], net)
    _sort_net(wires[h:], net)
    _oe_merge(wires[:h], wires[h:], net)


def sorting_network(n=N):
    net = []
    _sort_net(list(range(n)), net)
    return net


def _build_nc(p_core=P_CORE, fb=FB, nt=NT):
    import concourse.bacc as bacc
    import concourse.mybir as mybir
    from concourse.tile import TileContext
    from concourse.ap import AP

    F32 = mybir.dt.float32
    F16 = mybir.dt.float16
    Copy = mybir.ActivationFunctionType.Copy
    FBl, NTl, PTl = fb, nt, 128 * fb
    assert p_core == PTl * NTl
    net = sorting_network(N)

    nc = bacc.Bacc()
    fc = nc.declare_dram_parameter("forecasts", [N, p_core], F32, isOutput=False)
    tg = nc.declare_dram_parameter("target", [p_core], F32, isOutput=False)
    out = nc.declare_dram_parameter("partials", [128, CPT * NTl], F32, isOutput=True)

    with TileContext(nc) as tc:
        with (
            tc.tile_pool(name="io", bufs=2) as iop,
            tc.tile_pool(name="wk", bufs=2) as wkp,
            tc.tile_pool(name="scr", bufs=1) as scrp,
            tc.tile_pool(name="acc", bufs=1) as accp,
        ):
            accbuf = accp.tile([128, CPT * NTl], F32)
            dumscr = scrp.tile([128, FBl], F16)

            for t in range(NTl):
                p0, c0 = t * PTl, t * CPT
                buf = wkp.tile([128, (N + 1) * FBl], F16, tag="buf")
                yb = wkp.tile([128, FBl], F16, tag="yb")

                # load + convert x in chunks of CVT_CHUNK sample blocks
                for ch in range(N // CVT_CHUNK):
                    i0 = ch * CVT_CHUNK
                    xt = iop.tile([128, CVT_CHUNK * FBl], F32, tag="x32")
                    fap = fc[:, :]
                    src = AP(fap.tensor, i0 * p_core + p0,
                             [[FBl, 128], [p_core, CVT_CHUNK], [1, FBl]])
                    nc.sync.dma_start(
                        xt.rearrange("p (n f) -> p n f", n=CVT_CHUNK), src)
                    nc.scalar.activation(
                        buf[:, i0 * FBl:(i0 + CVT_CHUNK) * FBl], xt, Copy,
                        accum_out=accbuf[:, c0 + N + NMN + ch:c0 + N + NMN + ch + 1])
                yt = iop.tile([128, FBl], F32, tag="y32")
                nc.sync.dma_start(
                    yt, tg[p0:p0 + PTl].rearrange("(p f) -> p f", p=128))
                nc.scalar.activation(yb, yt, Copy,
                                     accum_out=accbuf[:, c0 + CPT - 1:c0 + CPT])

                # Mn: min(x_i, y) in chunks, summed on ScalarE
                yap = yb[:, :]
                for mc in range(NMN):
                    i0 = mc * MN_CHUNK
                    mnscr = wkp.tile([128, MN_CHUNK * FBl], F16, tag="mnscr")
                    yb3 = AP(yap.tensor, yap.offset,
                             [list(yap.ap[0]), [0, MN_CHUNK], list(yap.ap[1])])
                    nc.vector.tensor_tensor(
                        out=mnscr.rearrange("p (n f) -> p n f", n=MN_CHUNK),
                        in0=buf[:, i0 * FBl:(i0 + MN_CHUNK) * FBl]
                            .rearrange("p (n f) -> p n f", n=MN_CHUNK),
                        in1=yb3,
                        op=mybir.AluOpType.min)
                    nc.scalar.activation(
                        mnscr[:, :], mnscr[:, :], Copy,
                        accum_out=accbuf[:, c0 + N + mc:c0 + N + mc + 1])

                # sort the 20 blocks with the comparator network
                slot = list(range(N))
                free = N
                for (i, j) in net:
                    si, sj = slot[i], slot[j]
                    a = buf[:, si * FBl:(si + 1) * FBl]
                    b = buf[:, sj * FBl:(sj + 1) * FBl]
                    f = buf[:, free * FBl:(free + 1) * FBl]
                    nc.vector.tensor_tensor(out=f, in0=a, in1=b,
                                            op=mybir.AluOpType.max)
                    nc.vector.tensor_tensor(out=a, in0=a, in1=b,
                                            op=mybir.AluOpType.min)
                    slot[j] = free
                    free = sj

                # per-order-statistic sums (DVE tensor_scalar accum, f16)
                for k in range(N):
                    sk = slot[k]
                    nc.vector.tensor_scalar(
                        out=dumscr,
                        in0=buf[:, sk * FBl:(sk + 1) * FBl],
                        scalar1=1.0, scalar2=None,
                        op0=mybir.AluOpType.mult,
                        op1=mybir.AluOpType.add,
                        accum_out=accbuf[:, c0 + k:c0 + k + 1])

            nc.sync.dma_start(out[:, :], accbuf[:, :])
    nc.compile()
    return nc


def _combine(partials_list):
    """partials cols per tile: [0:20] sorted col sums, [20:20+NMN] mn,
    [20+NMN:20+NMN+NCVT] Sx, [-1] Sy."""
    coef = 2.0 * np.arange(N) - (N - 1)
    tot = 0.0
    for p in partials_list:
        p = np.asarray(p, dtype=np.float64).reshape(128, NT, CPT)
        cs = p[:, :, 0:N].sum(axis=(0, 1))          # per-k column sums
        Mn = p[:, :, N:N + NMN].sum()
        S = p[:, :, N + NMN:N + NMN + NCVT].sum()
        Y = p[:, :, CPT - 1].sum()
        PW = (coef * cs).sum()
        FT = S + N * Y - 2.0 * Mn
        tot += FT / N - PW / (N * N)
    return tot / P_TOTAL


def _run(forecasts, target, trace=False):
    from concourse.bass_utils import run_bass_kernel_spmd

    nc = _CACHE.get("nc")
    if nc is None:
        nc = _build_nc()
        _CACHE["nc"] = nc

    fcf = np.asarray(forecasts, dtype=np.float32).reshape(N, P_TOTAL)
    tgf = np.asarray(target, dtype=np.float32).reshape(P_TOTAL)
    in_maps = []
    for c in range(N_CORES):
        sl = slice(c * P_CORE, (c + 1) * P_CORE)
        in_maps.append({
            "forecasts": np.ascontiguousarray(fcf[:, sl]),
            "target": np.ascontiguousarray(tgf[sl]),
        })
    res = run_bass_kernel_spmd(nc, in_maps, list(range(N_CORES)), trace=trace)
    val = _combine([r["partials"] for r in res.results])
    return np.array(val, dtype=np.float32), res


def kernel(forecasts, target):
    val, _ = _run(forecasts, target)
    return val
